# revision 23
# baseline (speedup 1.0000x reference)
"""Trainium2 Bass kernel for BiAttention (b=8, n=m=1024, d=512).

Sharding: data-parallel over batch — one batch element per NeuronCore,
8 cores, no cross-core communication.

v2 design (all matmul operands bf16; rel-err budget is 2e-2, bf16 lands ~3e-3):

  x1T  (d,n) = transpose(x1)                    [PE transpose, bf16]
  x2Tw (d,m) = transpose(x2) * w3  ++ col m=w1  [w3 folded in; extra w1 col]
  sim chunk  = x1T_t^T @ x2Tw                   -> psum cols [m | s1]
               + logm1 row accumulated onto the s1 column (mask bias)
  E = exp(psum)        (n, vm+1) bf16; col vm = exp(s1+logm1) = g1  (free!)
  ET = transpose(E[:, :vm])                     [PE transpose, bf16]
  x1g = x1 * g1  ++ col d=g1   (g1 folded into U_col rhs; den2 = g-column)
  x2g = x2 * g2  ++ col d=g2   (g2 folded into U_row rhs; den1 = g-column)
  U_col u = E_u^T @ x1g   -> (m, d | den2); Q2C = U_col * (g2/den2)
  U_row t = ET_t^T @ x2g  -> (n, d | den1); c2q = U_row / den1
  V t     = ET_t^T @ Q2C  -> (n, d);  q2c_att = V / den1
  out = [x1, c2q, x1*c2q, x1*q2c_att]           (n, 4d) f32

exp(sim) needs no row/col scaling at all: softmax weights live in the tiny
per-tile column scales folded into the rhs operands and evictions.  The
row/col softmax denominators fall out of the contraction psums as the g
columns (split 256|257 so no matmul output crosses a 2KB PSUM bank).

Mask-suffix specialization: tiles of 128 that are fully masked at the end
of either sequence are skipped in the contractions; the host dispatches to
a NEFF compiled for (kn, km) kept-tile counts.  Partially-masked tiles are
exact via the exponent biases (g=0 rows contribute nothing).
"""

import numpy as np
from contextlib import ExitStack

import concourse.bacc as bacc
import concourse.tile as tile
import concourse.mybir as mybir
from concourse.bass_utils import run_bass_kernel_spmd
from concourse.masks import make_identity

F32 = mybir.dt.float32
BF = mybir.dt.bfloat16
U8 = mybir.dt.uint8
EXP = mybir.ActivationFunctionType.Exp
COPY = mybir.ActivationFunctionType.Copy

P = 128
N = 1024          # x1 rows
M = 1024          # x2 rows
D = 512           # feature dim
NT, MT, DC = N // P, M // P, D // P
NEGB = -30000.0   # exp(x + NEGB) == 0.0 exactly for |x| < 80

N_CORES = 8

_CACHE = {}


def _chunks(width, lim=512):
    out = []
    o = 0
    while o < width:
        w = min(lim, width - o)
        out.append((o, w))
        o += w
    return out


def _build(kn, km):
    """Build the kernel keeping the first kn n-tiles / km m-tiles of the
    contractions (tiles beyond that must be fully masked)."""
    vm = km * P  # valid m extent
    nc = bacc.Bacc("TRN2", target_bir_lowering=False, debug=False)
    x1d = nc.dram_tensor("x1", [N, D], F32, kind="ExternalInput").ap()
    x2d = nc.dram_tensor("x2", [M, D], F32, kind="ExternalInput").ap()
    m1d = nc.dram_tensor("x1_mask", [N], U8, kind="ExternalInput").ap()
    m2d = nc.dram_tensor("x2_mask", [M], U8, kind="ExternalInput").ap()
    wd = nc.dram_tensor("W", [3 * D], F32, kind="ExternalInput").ap()
    outd = nc.dram_tensor("out", [N, 4 * D], F32, kind="ExternalOutput").ap()

    x1r_d = x1d.rearrange("(t p) d -> p t d", p=P)
    x2r_d = x2d.rearrange("(t p) d -> p t d", p=P)
    out_r = outd.rearrange("(t p) e -> p t e", p=P)

    # sim psum chunks over the widened (vm+1) extent; last chunk carries s1
    mch = _chunks(vm + 1)

    with tile.TileContext(nc) as tc, ExitStack() as ctx:
        const = ctx.enter_context(tc.tile_pool(name="const", bufs=1))
        big = ctx.enter_context(tc.tile_pool(name="big", bufs=1))
        rows = ctx.enter_context(tc.tile_pool(name="rows", bufs=1))
        work = ctx.enter_context(tc.tile_pool(name="work", bufs=3))
        ps512 = ctx.enter_context(tc.tile_pool(name="ps512", bufs=2, space="PSUM"))
        ps257 = ctx.enter_context(tc.tile_pool(name="ps257", bufs=2, space="PSUM"))
        ps256 = ctx.enter_context(tc.tile_pool(name="ps256", bufs=2, space="PSUM"))
        pstb = ctx.enter_context(tc.tile_pool(name="pstb", bufs=2, space="PSUM"))

        # ---------- big buffers ----------
        x1n = big.tile([P, NT, D], F32)        # natural x1 (outputs)
        x2n = big.tile([P, km, D], F32)        # natural x2
        x1c = big.tile([P, NT, D], BF)         # bf16 x1 (transpose source)
        x2c = big.tile([P, km, D], BF)         # bf16 x2 (transpose source)
        x1g = big.tile([P, kn, D + 1], BF)     # x1*g1 ++ g1 col
        x2g = big.tile([P, km, D + 1], BF)     # x2*g2 ++ g2 col
        x1T = big.tile([P, DC, N], BF)         # (d_chunk, n)
        x2Tw = big.tile([P, DC, vm + 1], BF)   # (d_chunk, m)*w3 ++ w1 col
        E = big.tile([P, NT, vm + 1], BF)      # exp(sim); col vm = g1
        ET = big.tile([P, km, N], BF)          # E^T
        Q2C = big.tile([P, km, D], BF)         # q2c * g2

        # ---------- identity FIRST: the gpsimd queue must not make the
        # PE transposes wait behind anything ----------
        ident = const.tile([P, P], F32)
        make_identity(nc, ident)

        # ---------- input DMAs, ALL on the Act HWDGE queue in strict
        # need-order: the shared DMA engines drain one queue FIFO, so
        # completion order == need order, in half-quad granules.  (Putting
        # quads on two queues makes everything finish simultaneously at
        # ~17us; FIFO gets the first pair in at ~10us.) ----------
        wrow = rows.tile([1, 12 * P], F32)
        nc.scalar.dma_start(wrow[:], wd.rearrange("(a n) -> a n", a=1))
        npair = (NT + 1) // 2
        mpair = (km + 1) // 2
        nc.scalar.dma_start(x1n[:, 0:2, :], x1r_d[:, 0:2, :])
        nc.scalar.dma_start(x1n[:, 2:4, :], x1r_d[:, 2:4, :])
        nc.scalar.dma_start(x2n[:, 0:2, :], x2r_d[:, 0:2, :])
        nc.scalar.dma_start(x2n[:, 2:min(4, km), :], x2r_d[:, 2:min(4, km), :])
        m1row = rows.tile([1, N], U8)
        nc.scalar.dma_start(m1row[:], m1d.rearrange("(a n) -> a n", a=1))
        m2row = rows.tile([1, M], U8)
        nc.scalar.dma_start(m2row[:], m2d.rearrange("(a n) -> a n", a=1))
        nc.scalar.dma_start(x1n[:, 4:6, :], x1r_d[:, 4:6, :])
        nc.scalar.dma_start(x1n[:, 6:8, :], x1r_d[:, 6:8, :])
        for p in range(2, mpair):
            hi = min(2 * p + 2, km)
            nc.scalar.dma_start(x2n[:, 2 * p:hi, :], x2r_d[:, 2 * p:hi, :])

        identb = const.tile([P, P], BF)
        nc.vector.tensor_copy(identb[:], ident[:])
        onesb = const.tile([1, 1], BF)
        nc.vector.memset(onesb[:], 1.0)
        onef = const.tile([1, 1], F32)
        nc.vector.memset(onef[:], 1.0)
        # bf16 casts of the first x1 pair ahead of any PE-dependent DVE op
        nc.vector.tensor_copy(x1c[:, 0, :], x1n[:, 0, :])
        nc.vector.tensor_copy(x1c[:, 1, :], x1n[:, 1, :])

        # W row -> columns via PE row->col transposes
        pwc = ps256.tile([P, 12], F32, tag="ps256")
        for c in range(12):
            nc.tensor.transpose(pwc[:, c:c + 1], wrow[0:1, c * P:(c + 1) * P],
                                onef[0:1, 0:1])
        wcols = const.tile([P, 12], F32)  # (p, c): w1=0:4 w2=4:8 w3=8:12
        nc.vector.tensor_copy(wcols[:], pwc[:])
        w3rec = const.tile([P, 4], F32)
        nc.vector.reciprocal(w3rec[:], wcols[:, 8:12])
        u2r = const.tile([P, 4], BF)      # w2/w3 — recovers s2 from x2Tw
        nc.vector.tensor_mul(u2r[:], wcols[:, 4:8], w3rec[:])

        # ---------- x1/x2 transposes straight from the f32 naturals (no
        # cast dependency; evictions produce the bf16 operands), in pairs
        # matching the half-quad DMA granules ----------
        def x1_pair(p):
            for c in range(DC):
                pq = pstb.tile([P, 256], BF, tag="pst", name=f"x1p_{p}_{c}")
                for j in range(2):
                    nc.tensor.transpose(pq[:, j * P:(j + 1) * P],
                                        x1c[:, 2 * p + j, c * P:(c + 1) * P],
                                        identb[:])
                nc.vector.tensor_copy(x1T[:, c, p * 256:(p + 1) * 256], pq[:])

        def x2_pair(p):
            jw = min(2, km - 2 * p)
            for c in range(DC):
                pq = pstb.tile([P, 256], BF, tag="pst", name=f"x2p_{p}_{c}")
                for j in range(jw):
                    nc.tensor.transpose(pq[:, j * P:(j + 1) * P],
                                        x2c[:, 2 * p + j, c * P:(c + 1) * P],
                                        identb[:])
                # evict fused with w3 scaling (per-partition in (d, m) layout)
                nc.scalar.activation(x2Tw[:, c, p * 256:p * 256 + jw * P],
                                     pq[:, 0:jw * P], COPY,
                                     scale=wcols[:, 8 + c:9 + c])

        def w1_cols():
            for c in range(DC):
                nc.vector.tensor_copy(x2Tw[:, c, vm:vm + 1],
                                      wcols[:, c:c + 1])

        # ---------- sim -> E (exp evict; s1 col gains mask bias) ----------
        def sim_tile(t, h):
            off, w = mch[h]
            last = off + w == vm + 1
            pool = ps512 if w > 320 else ps257
            pe = pool.tile([P, w], F32, tag=pool.name, name=f"pe_{t}_{h}")
            for c in range(DC):
                nc.tensor.matmul(pe[:],
                                 x1T[:, c, t * P:(t + 1) * P],
                                 x2Tw[:, c, off:off + w],
                                 start=(c == 0), stop=(c == DC - 1 and not last))
            if last:
                # += logm1 on the s1 column only (PE row->col via 1-wide matmul)
                nc.tensor.matmul(pe[:, w - 1:w],
                                 logm1b[0:1, t * P:(t + 1) * P],
                                 onesb[0:1, 0:1],
                                 start=False, stop=True, skip_group_check=True)
            nc.scalar.activation(E[:, t, off:off + w], pe[:], EXP)

        g1c = const.tile([P, NT], F32)

        def x1_gate(t):
            # x1g = x1 * g1 ++ g1 col (fused scale + f32->bf16 cast)
            g1 = g1c[:, t:t + 1]
            nc.vector.tensor_copy(g1, E[:, t, vm:vm + 1])
            nc.vector.tensor_scalar_mul(x1g[:, t, 0:D], x1n[:, t, :], g1)
            nc.vector.tensor_copy(x1g[:, t, D:D + 1], g1)

        # ---------- s2/g2 path, then x2g ----------
        g2c = const.tile([P, km], F32)

        def s2_g2():
            brow = rows.tile([1, vm], F32)
            for h, (off, w) in enumerate(_chunks(vm)):
                ps_s = ps512.tile([1, w], F32, tag="ps512", name=f"ps_b2_{h}")
                for c in range(DC):
                    nc.tensor.matmul(ps_s[:], u2r[:, c:c + 1],
                                     x2Tw[:, c, off:off + w],
                                     start=(c == 0), stop=(c == DC - 1))
                nc.vector.tensor_add(brow[:, off:off + w], ps_s[:],
                                     logm2[:, off:off + w])
            pbc = ps256.tile([P, km], F32, tag="ps256", name="pbc")
            for k in range(km):
                nc.tensor.transpose(pbc[:, k:k + 1],
                                    brow[0:1, k * P:(k + 1) * P],
                                    onef[0:1, 0:1])
            nc.scalar.activation(g2c[:], pbc[:], EXP)

        def x2_gate(k):
            nc.vector.tensor_scalar_mul(x2g[:, k, 0:D], x2n[:, k, :],
                                        g2c[:, k:k + 1])
            nc.vector.tensor_copy(x2g[:, k, D:D + 1], g2c[:, k:k + 1])

        # ---------- E transposes -> ET via the DMA transpose XBAR ----------
        def e_xpose(t):
            # ET[:, u, t*128+j] = E[j, t, u*128+p] for all u at once
            nc.sync.dma_start(ET[:, 0:km, t * P:(t + 1) * P],
                              E[:, t, 0:vm], transpose=True)

        # ---------- schedule: sim + transposes interleaved ----------
        x1_pair(0)
        nc.vector.tensor_copy(x1c[:, 2, :], x1n[:, 2, :])
        nc.vector.tensor_copy(x1c[:, 3, :], x1n[:, 3, :])
        x1_pair(1)
        for k in range(min(4, km)):
            nc.vector.tensor_copy(x2c[:, k, :], x2n[:, k, :])
        x2_pair(0)
        x2_pair(1)
        # masks -> exponent-offset rows (0 valid / NEGB padded); after the
        # early casts so the DVE queue never stalls the PE transposes
        logm1b = rows.tile([1, N], BF)
        nc.vector.tensor_scalar_mul(logm1b[:], m1row[:], NEGB)
        logm2 = rows.tile([1, vm], F32)
        nc.vector.tensor_scalar_mul(logm2[:], m2row[0:1, 0:vm], NEGB)
        w1_cols()
        for t in (0, 1, 2, 3):
            sim_tile(t, 0)
        for t in (4, 5, 6, 7):
            nc.vector.tensor_copy(x1c[:, t, :], x1n[:, t, :])
        x1_pair(2)
        x1_pair(3)
        for k in range(4, km):
            nc.vector.tensor_copy(x2c[:, k, :], x2n[:, k, :])
        for p in range(2, (km + 1) // 2):
            x2_pair(p)
        for h in range(1, len(mch)):
            for t in (0, 1, 2, 3):
                sim_tile(t, h)
        # out block 0 = x1: one bulk store, now that x1n is fully loaded
        nc.sync.dma_start(out_r[:, :, 0:D], x1n[:])
        s2_g2()
        for h in range(len(mch)):
            sim_tile(4, h)
        for t in range(min(4, kn)):
            x1_gate(t)
        for k in range(km):
            x2_gate(k)
        for t in range(0, 4):
            e_xpose(t)
        for h in range(len(mch)):
            sim_tile(5, h)
        e_xpose(4)
        for h in range(len(mch)):
            sim_tile(6, h)
        e_xpose(5)
        for h in range(len(mch)):
            sim_tile(7, h)
        e_xpose(6)
        e_xpose(7)
        for t in range(4, kn):
            x1_gate(t)

        # ---------- U_col -> Q2C (scaled by g2/den2) ----------
        def u_col(u):
            pa = ps256.tile([P, 256], F32, tag="ps256", name=f"ua_{u}")
            pb = ps257.tile([P, 257], F32, tag="ps257", name=f"ub_{u}")
            for k in range(kn):
                lhs = E[:, k, u * P:(u + 1) * P]
                nc.tensor.matmul(pa[:], lhs, x1g[:, k, 0:256],
                                 start=(k == 0), stop=(k == kn - 1))
                nc.tensor.matmul(pb[:], lhs, x1g[:, k, 256:513],
                                 start=(k == 0), stop=(k == kn - 1))
            rg = work.tile([P, 1], F32, tag="rg", name=f"rg_{u}")
            nc.vector.reciprocal(rg[:], pb[:, 256:257])
            nc.vector.tensor_mul(rg[:], rg[:], g2c[:, u:u + 1])
            nc.scalar.activation(Q2C[:, u, 0:256], pa[:], COPY, scale=rg[:])
            nc.scalar.activation(Q2C[:, u, 256:512], pb[:, 0:256], COPY,
                                 scale=rg[:])

        for u in range(km):
            u_col(u)

        # ---------- U_row -> c2q ; out blocks 1, 2 ----------
        rden1c = const.tile([P, NT], F32)

        def u_row(t):
            pa = ps256.tile([P, 256], F32, tag="ps256", name=f"ra_{t}")
            pb = ps257.tile([P, 257], F32, tag="ps257", name=f"rb_{t}")
            for k in range(km):
                lhs = ET[:, k, t * P:(t + 1) * P]
                nc.tensor.matmul(pa[:], lhs, x2g[:, k, 0:256],
                                 start=(k == 0), stop=(k == km - 1))
                nc.tensor.matmul(pb[:], lhs, x2g[:, k, 256:513],
                                 start=(k == 0), stop=(k == km - 1))
            rd = rden1c[:, t:t + 1]
            nc.vector.reciprocal(rd, pb[:, 256:257])
            combo = work.tile([P, 2 * D], F32, tag="ev", name=f"cb_{t}")
            nc.scalar.activation(combo[:, 0:256], pa[:], COPY, scale=rd)
            nc.scalar.activation(combo[:, 256:512], pb[:, 0:256], COPY,
                                 scale=rd)
            nc.vector.tensor_mul(combo[:, D:2 * D], x1n[:, t, :], combo[:, 0:D])
            nc.sync.dma_start(out_r[:, t, D:3 * D], combo[:])

        # ---------- V -> q2c_att ; out block 3 = x1 . (V*rden1) ----------
        def v_row(t):
            pv = ps512.tile([P, D], F32, tag="ps512", name=f"pv_{t}")
            for k in range(km):
                nc.tensor.matmul(pv[:], ET[:, k, t * P:(t + 1) * P],
                                 Q2C[:, k, :],
                                 start=(k == 0), stop=(k == km - 1))
            vtmp = work.tile([P, D], F32, tag="x1rd", name=f"vt_{t}")
            nc.scalar.activation(vtmp[:], pv[:], COPY, scale=rden1c[:, t:t + 1])
            prod = work.tile([P, D], F32, tag="x1rd", name=f"pv2_{t}")
            eng = nc.gpsimd if t < NT - 2 else nc.vector
            eng.tensor_mul(prod[:], vtmp[:], x1n[:, t, :])
            nc.sync.dma_start(out_r[:, t, 3 * D:4 * D], prod[:])

        # interleave: V(t) only needs rden1c[t] (from u_row(t)) and Q2C
        u_row(0)
        for t in range(1, NT):
            u_row(t)
            v_row(t - 1)
        v_row(NT - 1)

    nc.compile()
    return nc


def _kept_tiles(mask):
    """Tiles (of 128) up to and including the last one with any valid row."""
    valid = ~mask.astype(bool)           # (b, L)
    any_valid = valid.reshape(valid.shape[0], -1, P).any(axis=2).any(axis=0)
    nz = np.nonzero(any_valid)[0]
    return int(nz[-1]) + 1 if len(nz) else 1


def _get_nc(kn, km):
    key = (kn, km)
    if key not in _CACHE:
        _CACHE[key] = _build(kn, km)
    return _CACHE[key]


def _run(inputs, trace=False, trace_cores=None):
    x1 = np.ascontiguousarray(np.asarray(inputs["x1"], dtype=np.float32))
    x2 = np.ascontiguousarray(np.asarray(inputs["x2"], dtype=np.float32))
    m1 = np.ascontiguousarray(np.asarray(inputs["x1_mask"]).astype(np.uint8))
    m2 = np.ascontiguousarray(np.asarray(inputs["x2_mask"]).astype(np.uint8))
    W = np.ascontiguousarray(np.asarray(inputs["W"], dtype=np.float32))
    nc = _get_nc(_kept_tiles(m1), _kept_tiles(m2))
    in_maps = [
        {"x1": x1[i], "x2": x2[i], "x1_mask": m1[i], "x2_mask": m2[i], "W": W}
        for i in range(N_CORES)
    ]
    res = run_bass_kernel_spmd(nc, in_maps, core_ids=list(range(N_CORES)),
                               trace=trace, trace_cores=trace_cores)
    out = np.stack([res.results[i]["out"] for i in range(N_CORES)], axis=0)
    return out.astype(np.float32), res


def kernel(x1, x1_mask, x2, x2_mask, W, bias=None, **_kw):
    # bias is mathematically irrelevant: a global additive constant cancels in
    # both softmaxes, and every output term is softmax-weighted.
    out, _ = _run({"x1": x1, "x1_mask": x1_mask, "x2": x2, "x2_mask": x2_mask,
                   "W": W})
    return out


# revision 24
# speedup vs baseline: 1.0185x; 1.0185x over previous
"""Trainium2 Bass kernel for BiAttention (b=8, n=m=1024, d=512).

Sharding: data-parallel over batch — one batch element per NeuronCore,
8 cores, no cross-core communication.

v2 design (all matmul operands bf16; rel-err budget is 2e-2, bf16 lands ~3e-3):

  x1T  (d,n) = transpose(x1)                    [PE transpose, bf16]
  x2Tw (d,m) = transpose(x2) * w3  ++ col m=w1  [w3 folded in; extra w1 col]
  sim chunk  = x1T_t^T @ x2Tw                   -> psum cols [m | s1]
               + logm1 row accumulated onto the s1 column (mask bias)
  E = exp(psum)        (n, vm+1) bf16; col vm = exp(s1+logm1) = g1  (free!)
  ET = transpose(E[:, :vm])                     [PE transpose, bf16]
  x1g = x1 * g1  ++ col d=g1   (g1 folded into U_col rhs; den2 = g-column)
  x2g = x2 * g2  ++ col d=g2   (g2 folded into U_row rhs; den1 = g-column)
  U_col u = E_u^T @ x1g   -> (m, d | den2); Q2C = U_col * (g2/den2)
  U_row t = ET_t^T @ x2g  -> (n, d | den1); c2q = U_row / den1
  V t     = ET_t^T @ Q2C  -> (n, d);  q2c_att = V / den1
  out = [x1, c2q, x1*c2q, x1*q2c_att]           (n, 4d) f32

exp(sim) needs no row/col scaling at all: softmax weights live in the tiny
per-tile column scales folded into the rhs operands and evictions.  The
row/col softmax denominators fall out of the contraction psums as the g
columns (split 256|257 so no matmul output crosses a 2KB PSUM bank).

Mask-suffix specialization: tiles of 128 that are fully masked at the end
of either sequence are skipped in the contractions; the host dispatches to
a NEFF compiled for (kn, km) kept-tile counts.  Partially-masked tiles are
exact via the exponent biases (g=0 rows contribute nothing).
"""

import numpy as np
from contextlib import ExitStack

import concourse.bacc as bacc
import concourse.tile as tile
import concourse.mybir as mybir
from concourse.bass_utils import run_bass_kernel_spmd
from concourse.masks import make_identity

F32 = mybir.dt.float32
BF = mybir.dt.bfloat16
U8 = mybir.dt.uint8
EXP = mybir.ActivationFunctionType.Exp
COPY = mybir.ActivationFunctionType.Copy

P = 128
N = 1024          # x1 rows
M = 1024          # x2 rows
D = 512           # feature dim
NT, MT, DC = N // P, M // P, D // P
NEGB = -30000.0   # exp(x + NEGB) == 0.0 exactly for |x| < 80

N_CORES = 8

_CACHE = {}


def _chunks(width, lim=512):
    out = []
    o = 0
    while o < width:
        w = min(lim, width - o)
        out.append((o, w))
        o += w
    return out


def _build(kn, km):
    """Build the kernel keeping the first kn n-tiles / km m-tiles of the
    contractions (tiles beyond that must be fully masked)."""
    vm = km * P  # valid m extent
    nc = bacc.Bacc("TRN2", target_bir_lowering=False, debug=False)
    x1d = nc.dram_tensor("x1", [N, D], F32, kind="ExternalInput").ap()
    x2d = nc.dram_tensor("x2", [M, D], F32, kind="ExternalInput").ap()
    m1d = nc.dram_tensor("x1_mask", [N], U8, kind="ExternalInput").ap()
    m2d = nc.dram_tensor("x2_mask", [M], U8, kind="ExternalInput").ap()
    wd = nc.dram_tensor("W", [3 * D], F32, kind="ExternalInput").ap()
    outd = nc.dram_tensor("out", [N, 4 * D], F32, kind="ExternalOutput").ap()

    x1r_d = x1d.rearrange("(t p) d -> p t d", p=P)
    x2r_d = x2d.rearrange("(t p) d -> p t d", p=P)
    out_r = outd.rearrange("(t p) e -> p t e", p=P)

    # sim psum chunks over the widened (vm+1) extent; last chunk carries s1
    mch = _chunks(vm + 1)

    with tile.TileContext(nc) as tc, ExitStack() as ctx:
        const = ctx.enter_context(tc.tile_pool(name="const", bufs=1))
        big = ctx.enter_context(tc.tile_pool(name="big", bufs=1))
        rows = ctx.enter_context(tc.tile_pool(name="rows", bufs=1))
        work = ctx.enter_context(tc.tile_pool(name="work", bufs=3))
        ps512 = ctx.enter_context(tc.tile_pool(name="ps512", bufs=2, space="PSUM"))
        ps257 = ctx.enter_context(tc.tile_pool(name="ps257", bufs=2, space="PSUM"))
        ps256 = ctx.enter_context(tc.tile_pool(name="ps256", bufs=2, space="PSUM"))
        pstb = ctx.enter_context(tc.tile_pool(name="pstb", bufs=2, space="PSUM"))

        # ---------- big buffers ----------
        x1n = big.tile([P, NT, D], F32)        # natural x1 (outputs)
        x2n = big.tile([P, km, D], F32)        # natural x2
        x1g = big.tile([P, kn, D + 1], BF)     # x1*g1 ++ g1 col
        x2g = big.tile([P, km, D + 1], BF)     # x2*g2 ++ g2 col
        x1T = big.tile([P, DC, N], BF)         # (d_chunk, n)
        x2Tw = big.tile([P, DC, vm + 1], BF)   # (d_chunk, m)*w3 ++ w1 col
        E = big.tile([P, NT, vm + 1], BF)      # exp(sim); col vm = g1
        ET = big.tile([P, km, N], BF)          # E^T
        Q2C = big.tile([P, km, D], BF)         # q2c * g2

        # ---------- identity FIRST: the gpsimd queue must not make the
        # PE transposes wait behind anything ----------
        ident = const.tile([P, P], F32)
        make_identity(nc, ident)

        # ---------- input DMAs, ALL on the Act HWDGE queue in strict
        # need-order: the shared DMA engines drain one queue FIFO, so
        # completion order == need order, in half-quad granules.  (Putting
        # quads on two queues makes everything finish simultaneously at
        # ~17us; FIFO gets the first pair in at ~10us.) ----------
        wrow = rows.tile([1, 12 * P], F32)
        nc.scalar.dma_start(wrow[:], wd.rearrange("(a n) -> a n", a=1))
        npair = (NT + 1) // 2
        mpair = (km + 1) // 2
        nc.scalar.dma_start(x1n[:, 0:2, :], x1r_d[:, 0:2, :])
        nc.scalar.dma_start(x1n[:, 2:4, :], x1r_d[:, 2:4, :])
        nc.scalar.dma_start(x2n[:, 0:2, :], x2r_d[:, 0:2, :])
        nc.scalar.dma_start(x2n[:, 2:min(4, km), :], x2r_d[:, 2:min(4, km), :])
        m1row = rows.tile([1, N], U8)
        nc.scalar.dma_start(m1row[:], m1d.rearrange("(a n) -> a n", a=1))
        m2row = rows.tile([1, M], U8)
        nc.scalar.dma_start(m2row[:], m2d.rearrange("(a n) -> a n", a=1))
        nc.scalar.dma_start(x1n[:, 4:6, :], x1r_d[:, 4:6, :])
        nc.scalar.dma_start(x1n[:, 6:8, :], x1r_d[:, 6:8, :])
        for p in range(2, mpair):
            hi = min(2 * p + 2, km)
            nc.scalar.dma_start(x2n[:, 2 * p:hi, :], x2r_d[:, 2 * p:hi, :])

        identb = const.tile([P, P], BF)
        nc.vector.tensor_copy(identb[:], ident[:])
        onesb = const.tile([1, 1], BF)
        nc.vector.memset(onesb[:], 1.0)
        onef = const.tile([1, 1], F32)
        nc.vector.memset(onef[:], 1.0)

        # W row -> columns via PE row->col transposes
        pwc = ps256.tile([P, 12], F32, tag="ps256")
        for c in range(12):
            nc.tensor.transpose(pwc[:, c:c + 1], wrow[0:1, c * P:(c + 1) * P],
                                onef[0:1, 0:1])
        wcols = const.tile([P, 12], F32)  # (p, c): w1=0:4 w2=4:8 w3=8:12
        nc.vector.tensor_copy(wcols[:], pwc[:])
        w3rec = const.tile([P, 4], F32)
        nc.vector.reciprocal(w3rec[:], wcols[:, 8:12])
        u2r = const.tile([P, 4], BF)      # w2/w3 — recovers s2 from x2Tw
        nc.vector.tensor_mul(u2r[:], wcols[:, 4:8], w3rec[:])

        # masks -> exponent-offset rows (0 valid / NEGB padded)
        logm1b = rows.tile([1, N], BF)
        nc.vector.tensor_scalar_mul(logm1b[:], m1row[:], NEGB)
        logm2 = rows.tile([1, vm], F32)
        nc.vector.tensor_scalar_mul(logm2[:], m2row[0:1, 0:vm], NEGB)

        # ---------- x1/x2 transposes straight from the f32 naturals (no
        # cast dependency; evictions produce the bf16 operands), in pairs
        # matching the half-quad DMA granules ----------
        def x1_pair(p):
            for c in range(DC):
                pq = pstb.tile([P, 256], F32, tag="pst", name=f"x1p_{p}_{c}")
                for j in range(2):
                    nc.tensor.transpose(pq[:, j * P:(j + 1) * P],
                                        x1n[:, 2 * p + j, c * P:(c + 1) * P],
                                        ident[:])
                nc.vector.tensor_copy(x1T[:, c, p * 256:(p + 1) * 256], pq[:])

        def x2_pair(p):
            jw = min(2, km - 2 * p)
            for c in range(DC):
                pq = pstb.tile([P, 256], F32, tag="pst", name=f"x2p_{p}_{c}")
                for j in range(jw):
                    nc.tensor.transpose(pq[:, j * P:(j + 1) * P],
                                        x2n[:, 2 * p + j, c * P:(c + 1) * P],
                                        ident[:])
                # evict fused with w3 scaling (per-partition in (d, m) layout)
                nc.scalar.activation(x2Tw[:, c, p * 256:p * 256 + jw * P],
                                     pq[:, 0:jw * P], COPY,
                                     scale=wcols[:, 8 + c:9 + c])

        def w1_cols():
            for c in range(DC):
                nc.vector.tensor_copy(x2Tw[:, c, vm:vm + 1],
                                      wcols[:, c:c + 1])

        # ---------- sim -> E (exp evict; s1 col gains mask bias) ----------
        def sim_tile(t, h):
            off, w = mch[h]
            last = off + w == vm + 1
            pool = ps512 if w > 320 else ps257
            pe = pool.tile([P, w], F32, tag=pool.name, name=f"pe_{t}_{h}")
            for c in range(DC):
                nc.tensor.matmul(pe[:],
                                 x1T[:, c, t * P:(t + 1) * P],
                                 x2Tw[:, c, off:off + w],
                                 start=(c == 0), stop=(c == DC - 1 and not last))
            if last:
                # += logm1 on the s1 column only (PE row->col via 1-wide matmul)
                nc.tensor.matmul(pe[:, w - 1:w],
                                 logm1b[0:1, t * P:(t + 1) * P],
                                 onesb[0:1, 0:1],
                                 start=False, stop=True, skip_group_check=True)
            nc.scalar.activation(E[:, t, off:off + w], pe[:], EXP)

        g1c = const.tile([P, NT], F32)

        def x1_gate(t):
            # x1g = x1 * g1 ++ g1 col (fused scale + f32->bf16 cast)
            g1 = g1c[:, t:t + 1]
            nc.vector.tensor_copy(g1, E[:, t, vm:vm + 1])
            nc.vector.tensor_scalar_mul(x1g[:, t, 0:D], x1n[:, t, :], g1)
            nc.vector.tensor_copy(x1g[:, t, D:D + 1], g1)

        # ---------- s2/g2 path, then x2g ----------
        g2c = const.tile([P, km], F32)

        def s2_g2():
            brow = rows.tile([1, vm], F32)
            for h, (off, w) in enumerate(_chunks(vm)):
                ps_s = ps512.tile([1, w], F32, tag="ps512", name=f"ps_b2_{h}")
                for c in range(DC):
                    nc.tensor.matmul(ps_s[:], u2r[:, c:c + 1],
                                     x2Tw[:, c, off:off + w],
                                     start=(c == 0), stop=(c == DC - 1))
                nc.vector.tensor_add(brow[:, off:off + w], ps_s[:],
                                     logm2[:, off:off + w])
            pbc = ps256.tile([P, km], F32, tag="ps256", name="pbc")
            for k in range(km):
                nc.tensor.transpose(pbc[:, k:k + 1],
                                    brow[0:1, k * P:(k + 1) * P],
                                    onef[0:1, 0:1])
            nc.scalar.activation(g2c[:], pbc[:], EXP)

        def x2_gate(k):
            nc.vector.tensor_scalar_mul(x2g[:, k, 0:D], x2n[:, k, :],
                                        g2c[:, k:k + 1])
            nc.vector.tensor_copy(x2g[:, k, D:D + 1], g2c[:, k:k + 1])

        # ---------- E transposes -> ET via the DMA transpose XBAR ----------
        def e_xpose(t):
            # ET[:, u, t*128+j] = E[j, t, u*128+p] for all u at once
            nc.sync.dma_start(ET[:, 0:km, t * P:(t + 1) * P],
                              E[:, t, 0:vm], transpose=True)

        # ---------- schedule: sim + transposes interleaved ----------
        x1_pair(0)
        x1_pair(1)
        x2_pair(0)
        x2_pair(1)
        w1_cols()
        for t in (0, 1, 2, 3):
            sim_tile(t, 0)
        x1_pair(2)
        x1_pair(3)
        for p in range(2, (km + 1) // 2):
            x2_pair(p)
        for h in range(1, len(mch)):
            for t in (0, 1, 2, 3):
                sim_tile(t, h)
        # out block 0 = x1: one bulk store, now that x1n is fully loaded
        nc.sync.dma_start(out_r[:, :, 0:D], x1n[:])
        s2_g2()
        for h in range(len(mch)):
            sim_tile(4, h)
        for t in range(min(4, kn)):
            x1_gate(t)
        for k in range(km):
            x2_gate(k)
        for t in range(0, 4):
            e_xpose(t)
        for h in range(len(mch)):
            sim_tile(5, h)
        e_xpose(4)
        for h in range(len(mch)):
            sim_tile(6, h)
        e_xpose(5)
        for h in range(len(mch)):
            sim_tile(7, h)
        e_xpose(6)
        e_xpose(7)
        for t in range(4, kn):
            x1_gate(t)

        # ---------- U_col -> Q2C (scaled by g2/den2) ----------
        def u_col(u):
            pa = ps256.tile([P, 256], F32, tag="ps256", name=f"ua_{u}")
            pb = ps257.tile([P, 257], F32, tag="ps257", name=f"ub_{u}")
            for k in range(kn):
                lhs = E[:, k, u * P:(u + 1) * P]
                nc.tensor.matmul(pa[:], lhs, x1g[:, k, 0:256],
                                 start=(k == 0), stop=(k == kn - 1))
                nc.tensor.matmul(pb[:], lhs, x1g[:, k, 256:513],
                                 start=(k == 0), stop=(k == kn - 1))
            rg = work.tile([P, 1], F32, tag="rg", name=f"rg_{u}")
            nc.vector.reciprocal(rg[:], pb[:, 256:257])
            nc.vector.tensor_mul(rg[:], rg[:], g2c[:, u:u + 1])
            nc.scalar.activation(Q2C[:, u, 0:256], pa[:], COPY, scale=rg[:])
            nc.scalar.activation(Q2C[:, u, 256:512], pb[:, 0:256], COPY,
                                 scale=rg[:])

        for u in range(km):
            u_col(u)

        # ---------- U_row -> c2q ; out blocks 1, 2 ----------
        rden1c = const.tile([P, NT], F32)

        def u_row(t):
            pa = ps256.tile([P, 256], F32, tag="ps256", name=f"ra_{t}")
            pb = ps257.tile([P, 257], F32, tag="ps257", name=f"rb_{t}")
            for k in range(km):
                lhs = ET[:, k, t * P:(t + 1) * P]
                nc.tensor.matmul(pa[:], lhs, x2g[:, k, 0:256],
                                 start=(k == 0), stop=(k == km - 1))
                nc.tensor.matmul(pb[:], lhs, x2g[:, k, 256:513],
                                 start=(k == 0), stop=(k == km - 1))
            rd = rden1c[:, t:t + 1]
            nc.vector.reciprocal(rd, pb[:, 256:257])
            combo = work.tile([P, 2 * D], F32, tag="ev", name=f"cb_{t}")
            nc.scalar.activation(combo[:, 0:256], pa[:], COPY, scale=rd)
            nc.scalar.activation(combo[:, 256:512], pb[:, 0:256], COPY,
                                 scale=rd)
            nc.vector.tensor_mul(combo[:, D:2 * D], x1n[:, t, :], combo[:, 0:D])
            nc.sync.dma_start(out_r[:, t, D:3 * D], combo[:])

        # ---------- V -> q2c_att ; out block 3 = x1 . (V*rden1) ----------
        def v_row(t):
            pv = ps512.tile([P, D], F32, tag="ps512", name=f"pv_{t}")
            for k in range(km):
                nc.tensor.matmul(pv[:], ET[:, k, t * P:(t + 1) * P],
                                 Q2C[:, k, :],
                                 start=(k == 0), stop=(k == km - 1))
            vtmp = work.tile([P, D], F32, tag="x1rd", name=f"vt_{t}")
            nc.scalar.activation(vtmp[:], pv[:], COPY, scale=rden1c[:, t:t + 1])
            prod = work.tile([P, D], F32, tag="x1rd", name=f"pv2_{t}")
            eng = nc.gpsimd if t < NT - 2 else nc.vector
            eng.tensor_mul(prod[:], vtmp[:], x1n[:, t, :])
            nc.sync.dma_start(out_r[:, t, 3 * D:4 * D], prod[:])

        # interleave: V(t) only needs rden1c[t] (from u_row(t)) and Q2C
        u_row(0)
        for t in range(1, NT):
            u_row(t)
            v_row(t - 1)
        v_row(NT - 1)

    nc.compile()
    return nc


def _kept_tiles(mask):
    """Tiles (of 128) up to and including the last one with any valid row."""
    valid = ~mask.astype(bool)           # (b, L)
    any_valid = valid.reshape(valid.shape[0], -1, P).any(axis=2).any(axis=0)
    nz = np.nonzero(any_valid)[0]
    return int(nz[-1]) + 1 if len(nz) else 1


def _get_nc(kn, km):
    key = (kn, km)
    if key not in _CACHE:
        _CACHE[key] = _build(kn, km)
    return _CACHE[key]


def _run(inputs, trace=False, trace_cores=None):
    x1 = np.ascontiguousarray(np.asarray(inputs["x1"], dtype=np.float32))
    x2 = np.ascontiguousarray(np.asarray(inputs["x2"], dtype=np.float32))
    m1 = np.ascontiguousarray(np.asarray(inputs["x1_mask"]).astype(np.uint8))
    m2 = np.ascontiguousarray(np.asarray(inputs["x2_mask"]).astype(np.uint8))
    W = np.ascontiguousarray(np.asarray(inputs["W"], dtype=np.float32))
    nc = _get_nc(_kept_tiles(m1), _kept_tiles(m2))
    in_maps = [
        {"x1": x1[i], "x2": x2[i], "x1_mask": m1[i], "x2_mask": m2[i], "W": W}
        for i in range(N_CORES)
    ]
    res = run_bass_kernel_spmd(nc, in_maps, core_ids=list(range(N_CORES)),
                               trace=trace, trace_cores=trace_cores)
    out = np.stack([res.results[i]["out"] for i in range(N_CORES)], axis=0)
    return out.astype(np.float32), res


def kernel(x1, x1_mask, x2, x2_mask, W, bias=None, **_kw):
    # bias is mathematically irrelevant: a global additive constant cancels in
    # both softmaxes, and every output term is softmax-weighted.
    out, _ = _run({"x1": x1, "x1_mask": x1_mask, "x2": x2, "x2_mask": x2_mask,
                   "W": W})
    return out


# revision 25
# speedup vs baseline: 1.0674x; 1.0481x over previous
"""Trainium2 Bass kernel for BiAttention (b=8, n=m=1024, d=512).

Sharding: data-parallel over batch — one batch element per NeuronCore,
8 cores, no cross-core communication.

v2 design (all matmul operands bf16; rel-err budget is 2e-2, bf16 lands ~3e-3):

  x1T  (d,n) = transpose(x1)                    [PE transpose, bf16]
  x2Tw (d,m) = transpose(x2) * w3  ++ col m=w1  [w3 folded in; extra w1 col]
  sim chunk  = x1T_t^T @ x2Tw                   -> psum cols [m | s1]
               + logm1 row accumulated onto the s1 column (mask bias)
  E = exp(psum)        (n, vm+1) bf16; col vm = exp(s1+logm1) = g1  (free!)
  ET = transpose(E[:, :vm])                     [PE transpose, bf16]
  x1g = x1 * g1  ++ col d=g1   (g1 folded into U_col rhs; den2 = g-column)
  x2g = x2 * g2  ++ col d=g2   (g2 folded into U_row rhs; den1 = g-column)
  U_col u = E_u^T @ x1g   -> (m, d | den2); Q2C = U_col * (g2/den2)
  U_row t = ET_t^T @ x2g  -> (n, d | den1); c2q = U_row / den1
  V t     = ET_t^T @ Q2C  -> (n, d);  q2c_att = V / den1
  out = [x1, c2q, x1*c2q, x1*q2c_att]           (n, 4d) f32

exp(sim) needs no row/col scaling at all: softmax weights live in the tiny
per-tile column scales folded into the rhs operands and evictions.  The
row/col softmax denominators fall out of the contraction psums as the g
columns (split 256|257 so no matmul output crosses a 2KB PSUM bank).

Mask-suffix specialization: tiles of 128 that are fully masked at the end
of either sequence are skipped in the contractions; the host dispatches to
a NEFF compiled for (kn, km) kept-tile counts.  Partially-masked tiles are
exact via the exponent biases (g=0 rows contribute nothing).
"""

import numpy as np
from contextlib import ExitStack

import concourse.bacc as bacc
import concourse.tile as tile
import concourse.mybir as mybir
from concourse.bass_utils import run_bass_kernel_spmd
from concourse.masks import make_identity

F32 = mybir.dt.float32
BF = mybir.dt.bfloat16
U8 = mybir.dt.uint8
EXP = mybir.ActivationFunctionType.Exp
COPY = mybir.ActivationFunctionType.Copy

P = 128
N = 1024          # x1 rows
M = 1024          # x2 rows
D = 512           # feature dim
NT, MT, DC = N // P, M // P, D // P
NEGB = -30000.0   # exp(x + NEGB) == 0.0 exactly for |x| < 80

N_CORES = 8

_CACHE = {}


def _chunks(width, lim=512):
    out = []
    o = 0
    while o < width:
        w = min(lim, width - o)
        out.append((o, w))
        o += w
    return out


def _build(kn, km):
    """Build the kernel keeping the first kn n-tiles / km m-tiles of the
    contractions (tiles beyond that must be fully masked)."""
    vm = km * P  # valid m extent
    nc = bacc.Bacc("TRN2", target_bir_lowering=False, debug=False)
    x1d = nc.dram_tensor("x1", [N, D], F32, kind="ExternalInput").ap()
    x2d = nc.dram_tensor("x2", [M, D], F32, kind="ExternalInput").ap()
    m1d = nc.dram_tensor("x1_mask", [N], U8, kind="ExternalInput").ap()
    m2d = nc.dram_tensor("x2_mask", [M], U8, kind="ExternalInput").ap()
    wd = nc.dram_tensor("W", [3 * D], F32, kind="ExternalInput").ap()
    outd = nc.dram_tensor("out", [N, 4 * D], F32, kind="ExternalOutput").ap()

    x1r_d = x1d.rearrange("(t p) d -> p t d", p=P)
    x2r_d = x2d.rearrange("(t p) d -> p t d", p=P)
    out_r = outd.rearrange("(t p) e -> p t e", p=P)

    # sim psum chunks over the widened (vm+1) extent; last chunk carries s1
    mch = _chunks(vm + 1)

    with tile.TileContext(nc) as tc, ExitStack() as ctx:
        const = ctx.enter_context(tc.tile_pool(name="const", bufs=1))
        big = ctx.enter_context(tc.tile_pool(name="big", bufs=1))
        rows = ctx.enter_context(tc.tile_pool(name="rows", bufs=1))
        work = ctx.enter_context(tc.tile_pool(name="work", bufs=4))
        ps512 = ctx.enter_context(tc.tile_pool(name="ps512", bufs=2, space="PSUM"))
        ps257 = ctx.enter_context(tc.tile_pool(name="ps257", bufs=2, space="PSUM"))
        ps256 = ctx.enter_context(tc.tile_pool(name="ps256", bufs=2, space="PSUM"))
        pstb = ctx.enter_context(tc.tile_pool(name="pstb", bufs=2, space="PSUM"))

        # ---------- big buffers ----------
        x1n = big.tile([P, NT, D], F32)        # natural x1 (outputs)
        x2n = big.tile([P, km, D], F32)        # natural x2
        x1g = big.tile([P, kn, D + 1], BF)     # x1*g1 ++ g1 col
        x2g = big.tile([P, km, D + 1], BF)     # x2*g2 ++ g2 col
        x1T = big.tile([P, DC, N], BF)         # (d_chunk, n)
        x2Tw = big.tile([P, DC, vm + 1], BF)   # (d_chunk, m)*w3 ++ w1 col
        E = big.tile([P, NT, vm + 1], BF)      # exp(sim); col vm = g1
        ET = big.tile([P, km, N], BF)          # E^T
        Q2C = big.tile([P, km, D], BF)         # q2c * g2

        # ---------- identity FIRST: the gpsimd queue must not make the
        # PE transposes wait behind anything ----------
        ident = const.tile([P, P], F32)
        make_identity(nc, ident)

        # ---------- input DMAs, ALL on the Act HWDGE queue in strict
        # need-order: the shared DMA engines drain one queue FIFO, so
        # completion order == need order, in half-quad granules.  (Putting
        # quads on two queues makes everything finish simultaneously at
        # ~17us; FIFO gets the first pair in at ~10us.) ----------
        wrow = rows.tile([1, 12 * P], F32)
        nc.scalar.dma_start(wrow[:], wd.rearrange("(a n) -> a n", a=1))
        npair = (NT + 1) // 2
        mpair = (km + 1) // 2
        nc.scalar.dma_start(x1n[:, 0:2, :], x1r_d[:, 0:2, :])
        nc.scalar.dma_start(x1n[:, 2:4, :], x1r_d[:, 2:4, :])
        nc.scalar.dma_start(x2n[:, 0:2, :], x2r_d[:, 0:2, :])
        nc.scalar.dma_start(x2n[:, 2:min(4, km), :], x2r_d[:, 2:min(4, km), :])
        m1row = rows.tile([1, N], U8)
        nc.scalar.dma_start(m1row[:], m1d.rearrange("(a n) -> a n", a=1))
        m2row = rows.tile([1, M], U8)
        nc.scalar.dma_start(m2row[:], m2d.rearrange("(a n) -> a n", a=1))
        nc.scalar.dma_start(x1n[:, 4:6, :], x1r_d[:, 4:6, :])
        nc.scalar.dma_start(x1n[:, 6:8, :], x1r_d[:, 6:8, :])
        for p in range(2, mpair):
            hi = min(2 * p + 2, km)
            nc.scalar.dma_start(x2n[:, 2 * p:hi, :], x2r_d[:, 2 * p:hi, :])

        identb = const.tile([P, P], BF)
        nc.vector.tensor_copy(identb[:], ident[:])
        onesb = const.tile([1, 1], BF)
        nc.vector.memset(onesb[:], 1.0)
        onef = const.tile([1, 1], F32)
        nc.vector.memset(onef[:], 1.0)

        # W row -> columns via PE row->col transposes
        pwc = ps256.tile([P, 12], F32, tag="ps256")
        for c in range(12):
            nc.tensor.transpose(pwc[:, c:c + 1], wrow[0:1, c * P:(c + 1) * P],
                                onef[0:1, 0:1])
        wcols = const.tile([P, 12], F32)  # (p, c): w1=0:4 w2=4:8 w3=8:12
        nc.vector.tensor_copy(wcols[:], pwc[:])
        w3rec = const.tile([P, 4], F32)
        nc.vector.reciprocal(w3rec[:], wcols[:, 8:12])
        u2r = const.tile([P, 4], BF)      # w2/w3 — recovers s2 from x2Tw
        nc.vector.tensor_mul(u2r[:], wcols[:, 4:8], w3rec[:])

        # masks -> exponent-offset rows (0 valid / NEGB padded)
        logm1b = rows.tile([1, N], BF)
        nc.vector.tensor_scalar_mul(logm1b[:], m1row[:], NEGB)
        logm2 = rows.tile([1, vm], F32)
        nc.vector.tensor_scalar_mul(logm2[:], m2row[0:1, 0:vm], NEGB)

        # ---------- x1/x2 transposes straight from the f32 naturals (no
        # cast dependency; evictions produce the bf16 operands), in pairs
        # matching the half-quad DMA granules ----------
        def x1_pair(p):
            for c in range(DC):
                pq = pstb.tile([P, 256], F32, tag="pst", name=f"x1p_{p}_{c}")
                for j in range(2):
                    nc.tensor.transpose(pq[:, j * P:(j + 1) * P],
                                        x1n[:, 2 * p + j, c * P:(c + 1) * P],
                                        ident[:])
                nc.vector.tensor_copy(x1T[:, c, p * 256:(p + 1) * 256], pq[:])

        def x2_pair(p):
            jw = min(2, km - 2 * p)
            for c in range(DC):
                pq = pstb.tile([P, 256], F32, tag="pst", name=f"x2p_{p}_{c}")
                for j in range(jw):
                    nc.tensor.transpose(pq[:, j * P:(j + 1) * P],
                                        x2n[:, 2 * p + j, c * P:(c + 1) * P],
                                        ident[:])
                # evict fused with w3 scaling (per-partition in (d, m) layout)
                nc.scalar.activation(x2Tw[:, c, p * 256:p * 256 + jw * P],
                                     pq[:, 0:jw * P], COPY,
                                     scale=wcols[:, 8 + c:9 + c])

        def w1_cols():
            for c in range(DC):
                nc.vector.tensor_copy(x2Tw[:, c, vm:vm + 1],
                                      wcols[:, c:c + 1])

        # ---------- sim -> E (exp evict; s1 col gains mask bias) ----------
        def sim_tile(t, h):
            off, w = mch[h]
            last = off + w == vm + 1
            pool = ps512 if w > 320 else ps257
            pe = pool.tile([P, w], F32, tag=pool.name, name=f"pe_{t}_{h}")
            for c in range(DC):
                nc.tensor.matmul(pe[:],
                                 x1T[:, c, t * P:(t + 1) * P],
                                 x2Tw[:, c, off:off + w],
                                 start=(c == 0), stop=(c == DC - 1 and not last))
            if last:
                # += logm1 on the s1 column only (PE row->col via 1-wide matmul)
                nc.tensor.matmul(pe[:, w - 1:w],
                                 logm1b[0:1, t * P:(t + 1) * P],
                                 onesb[0:1, 0:1],
                                 start=False, stop=True, skip_group_check=True)
            nc.scalar.activation(E[:, t, off:off + w], pe[:], EXP)

        g1c = const.tile([P, NT], F32)

        def x1_gate(t):
            # x1g = x1 * g1 ++ g1 col (fused scale + f32->bf16 cast)
            g1 = g1c[:, t:t + 1]
            nc.vector.tensor_copy(g1, E[:, t, vm:vm + 1])
            nc.vector.tensor_scalar_mul(x1g[:, t, 0:D], x1n[:, t, :], g1)
            nc.vector.tensor_copy(x1g[:, t, D:D + 1], g1)

        # ---------- s2/g2 path, then x2g ----------
        g2c = const.tile([P, km], F32)

        def s2_g2():
            brow = rows.tile([1, vm], F32)
            for h, (off, w) in enumerate(_chunks(vm)):
                ps_s = ps512.tile([1, w], F32, tag="ps512", name=f"ps_b2_{h}")
                for c in range(DC):
                    nc.tensor.matmul(ps_s[:], u2r[:, c:c + 1],
                                     x2Tw[:, c, off:off + w],
                                     start=(c == 0), stop=(c == DC - 1))
                nc.vector.tensor_add(brow[:, off:off + w], ps_s[:],
                                     logm2[:, off:off + w])
            pbc = ps256.tile([P, km], F32, tag="ps256", name="pbc")
            for k in range(km):
                nc.tensor.transpose(pbc[:, k:k + 1],
                                    brow[0:1, k * P:(k + 1) * P],
                                    onef[0:1, 0:1])
            nc.scalar.activation(g2c[:], pbc[:], EXP)

        def x2_gate(k):
            nc.vector.tensor_scalar_mul(x2g[:, k, 0:D], x2n[:, k, :],
                                        g2c[:, k:k + 1])
            nc.vector.tensor_copy(x2g[:, k, D:D + 1], g2c[:, k:k + 1])

        # ---------- E transposes -> ET via the DMA transpose XBAR ----------
        def e_xpose(t):
            # ET[:, u, t*128+j] = E[j, t, u*128+p] for all u at once
            nc.sync.dma_start(ET[:, 0:km, t * P:(t + 1) * P],
                              E[:, t, 0:vm], transpose=True)

        # ---------- schedule: sim + transposes interleaved ----------
        x1_pair(0)
        x1_pair(1)
        x2_pair(0)
        x2_pair(1)
        w1_cols()
        for t in (0, 1, 2, 3):
            sim_tile(t, 0)
        x1_pair(2)
        x1_pair(3)
        for p in range(2, (km + 1) // 2):
            x2_pair(p)
        for h in range(1, len(mch)):
            for t in (0, 1, 2, 3):
                sim_tile(t, h)
        # out block 0 = x1: one bulk store, now that x1n is fully loaded
        nc.sync.dma_start(out_r[:, :, 0:D], x1n[:])
        s2_g2()
        for h in range(len(mch)):
            sim_tile(4, h)
        for t in range(min(4, kn)):
            x1_gate(t)
        for k in range(km):
            x2_gate(k)
        for t in range(0, 4):
            e_xpose(t)
        for h in range(len(mch)):
            sim_tile(5, h)
        e_xpose(4)
        for h in range(len(mch)):
            sim_tile(6, h)
        e_xpose(5)
        for h in range(len(mch)):
            sim_tile(7, h)
        e_xpose(6)
        e_xpose(7)
        for t in range(4, kn):
            x1_gate(t)

        # ---------- U_col -> Q2C (scaled by g2/den2) ----------
        def u_col(u):
            pa = ps256.tile([P, 256], F32, tag="ps256", name=f"ua_{u}")
            pb = ps257.tile([P, 257], F32, tag="ps257", name=f"ub_{u}")
            for k in range(kn):
                lhs = E[:, k, u * P:(u + 1) * P]
                nc.tensor.matmul(pa[:], lhs, x1g[:, k, 0:256],
                                 start=(k == 0), stop=(k == kn - 1))
                nc.tensor.matmul(pb[:], lhs, x1g[:, k, 256:513],
                                 start=(k == 0), stop=(k == kn - 1))
            rg = work.tile([P, 1], F32, tag="rg", name=f"rg_{u}")
            nc.vector.reciprocal(rg[:], pb[:, 256:257])
            nc.vector.tensor_mul(rg[:], rg[:], g2c[:, u:u + 1])
            nc.scalar.activation(Q2C[:, u, 0:256], pa[:], COPY, scale=rg[:])
            nc.scalar.activation(Q2C[:, u, 256:512], pb[:, 0:256], COPY,
                                 scale=rg[:])

        for u in range(km):
            u_col(u)

        # ---------- U_row -> c2q ; out blocks 1, 2 ----------
        rden1c = const.tile([P, NT], F32)

        def u_row(t):
            pa = ps256.tile([P, 256], F32, tag="ps256", name=f"ra_{t}")
            pb = ps257.tile([P, 257], F32, tag="ps257", name=f"rb_{t}")
            for k in range(km):
                lhs = ET[:, k, t * P:(t + 1) * P]
                nc.tensor.matmul(pa[:], lhs, x2g[:, k, 0:256],
                                 start=(k == 0), stop=(k == km - 1))
                nc.tensor.matmul(pb[:], lhs, x2g[:, k, 256:513],
                                 start=(k == 0), stop=(k == km - 1))
            rd = rden1c[:, t:t + 1]
            nc.vector.reciprocal(rd, pb[:, 256:257])
            combo = work.tile([P, 2 * D], F32, tag="ev", name=f"cb_{t}")
            nc.scalar.activation(combo[:, 0:256], pa[:], COPY, scale=rd)
            nc.scalar.activation(combo[:, 256:512], pb[:, 0:256], COPY,
                                 scale=rd)
            nc.vector.tensor_mul(combo[:, D:2 * D], x1n[:, t, :], combo[:, 0:D])
            nc.sync.dma_start(out_r[:, t, D:3 * D], combo[:])

        # ---------- V -> q2c_att ; out block 3 = x1 . (V*rden1) ----------
        def v_row(t):
            pv = ps512.tile([P, D], F32, tag="ps512", name=f"pv_{t}")
            if t >= NT - 2:
                # tail tiles: precompute x1*rden1 on DVE DURING the matmuls,
                # then a single psum-read mul — drops the Act hop from the
                # critical tail chain
                x1rd = work.tile([P, D], F32, tag="x1rd", name=f"xr_{t}")
                nc.vector.tensor_scalar_mul(x1rd[:], x1n[:, t, :],
                                            rden1c[:, t:t + 1])
            for k in range(km):
                nc.tensor.matmul(pv[:], ET[:, k, t * P:(t + 1) * P],
                                 Q2C[:, k, :],
                                 start=(k == 0), stop=(k == km - 1))
            prod = work.tile([P, D], F32, tag="x1rd", name=f"pv2_{t}")
            if t >= NT - 2:
                nc.vector.tensor_mul(prod[:], x1rd[:], pv[:])
            else:
                vtmp = work.tile([P, D], F32, tag="x1rd", name=f"vt_{t}")
                nc.scalar.activation(vtmp[:], pv[:], COPY,
                                     scale=rden1c[:, t:t + 1])
                nc.gpsimd.tensor_mul(prod[:], vtmp[:], x1n[:, t, :])
            nc.sync.dma_start(out_r[:, t, 3 * D:4 * D], prod[:])

        # interleave: V(t) only needs rden1c[t] (from u_row(t)) and Q2C
        u_row(0)
        for t in range(1, NT):
            u_row(t)
            v_row(t - 1)
        v_row(NT - 1)

    nc.compile()
    return nc


def _kept_tiles(mask):
    """Tiles (of 128) up to and including the last one with any valid row."""
    valid = ~mask.astype(bool)           # (b, L)
    any_valid = valid.reshape(valid.shape[0], -1, P).any(axis=2).any(axis=0)
    nz = np.nonzero(any_valid)[0]
    return int(nz[-1]) + 1 if len(nz) else 1


def _get_nc(kn, km):
    key = (kn, km)
    if key not in _CACHE:
        _CACHE[key] = _build(kn, km)
    return _CACHE[key]


def _run(inputs, trace=False, trace_cores=None):
    x1 = np.ascontiguousarray(np.asarray(inputs["x1"], dtype=np.float32))
    x2 = np.ascontiguousarray(np.asarray(inputs["x2"], dtype=np.float32))
    m1 = np.ascontiguousarray(np.asarray(inputs["x1_mask"]).astype(np.uint8))
    m2 = np.ascontiguousarray(np.asarray(inputs["x2_mask"]).astype(np.uint8))
    W = np.ascontiguousarray(np.asarray(inputs["W"], dtype=np.float32))
    nc = _get_nc(_kept_tiles(m1), _kept_tiles(m2))
    in_maps = [
        {"x1": x1[i], "x2": x2[i], "x1_mask": m1[i], "x2_mask": m2[i], "W": W}
        for i in range(N_CORES)
    ]
    res = run_bass_kernel_spmd(nc, in_maps, core_ids=list(range(N_CORES)),
                               trace=trace, trace_cores=trace_cores)
    out = np.stack([res.results[i]["out"] for i in range(N_CORES)], axis=0)
    return out.astype(np.float32), res


def kernel(x1, x1_mask, x2, x2_mask, W, bias=None, **_kw):
    # bias is mathematically irrelevant: a global additive constant cancels in
    # both softmaxes, and every output term is softmax-weighted.
    out, _ = _run({"x1": x1, "x1_mask": x1_mask, "x2": x2, "x2_mask": x2_mask,
                   "W": W})
    return out
